# revision 1
# baseline (speedup 1.0000x reference)
"""Recursive LSTM decoder (T=512, B=512, I=128, H=512) on 8 trn2 NeuronCores.

Strategy: data-parallel over batch (64 rows/core, weights replicated, no
collectives). All on-chip state is kept in transposed layout
[feature-on-partition, batch-on-free] so the serial recurrence needs no
transposes. Matmul inputs are bf16 (1 cycle/row on PE), accumulation and
elementwise math are fp32; the cell state c stays fp32.

Per step (per core):
  gates.T[m-chunk 128, b 64] = sum_k Wcat.T-chunk(k,m) @ catT-chunk(k)
    (16 m-chunks x 5 k-chunks; PSUM grouped by output H-chunk so ACT/DVE of
     group c overlaps PE of group c+1)
  i,f,o = sigmoid(. + b), g = tanh(. + b)   (bias folded into ACT)
  c = f*c + i*g ; h = o*tanh(c)
  feedback: inT = tanh(0.5*(fcW.T-chunks @ hT) + fc_b/2)   [= 2*sigmoid(z)-1]
  output:   out[64,128] = tanh(0.5*(hT-chunks as stationary @ fcW-moving + fc_b))
  out -> DRAM at index (T-1-t)  (reference stores outputs reversed)

All constants+init are shipped in 2 bundled DMAs and the output store uses
the single SWDGE queue: per-instruction sync-wait fan-in must stay <= the
ISA cap (walrus "Too many sync wait commands" otherwise).
"""

import numpy as np
import ml_dtypes

import concourse.bass as bass
import concourse.mybir as mybir
import concourse.tile as tile
from concourse import bacc
from concourse.bass import ds
from concourse.expressions import smax
from concourse.bass_utils import run_bass_kernel_spmd

T, B, I, H = 512, 512, 128, 512
NCORES = 8
BS = B // NCORES          # 64 batch rows per core
HC = H // 128             # 4 h chunks
NM = (4 * H) // 128       # 16 gate m-chunks
NK = (I + H) // 128       # 5 cat k-chunks (1 input + 4 hidden)

# bf16 constant-bundle column offsets
OFF_WG = 0                       # [128, NM*NK*128] gate weight chunks
OFF_WFC = OFF_WG + NM * NK * 128  # [128, HC*128] fc weight chunks
OFF_XT = OFF_WFC + HC * 128      # [128, BS] x[T-1] transposed
OFF_H0 = OFF_XT + BS             # [128, HC*BS] h0 transposed
OFF_FCBR = OFF_H0 + HC * BS      # [1, 128] fc bias row (row 0 only)
CB_COLS = OFF_FCBR + 128
# f32 constant-bundle column offsets
OFF_BB = 0                       # [128, 4*HC*BS] gate bias broadcast (j,c,b)
OFF_FCBH = OFF_BB + 4 * HC * BS  # [128, 1] fc_b / 2
OFF_C0 = OFF_FCBH + 1            # [128, HC*BS] c0 transposed
CF_COLS = OFF_C0 + HC * BS

BF16 = mybir.dt.bfloat16
F32 = mybir.dt.float32
AF = mybir.ActivationFunctionType


def build(nsteps: int, out_steps: int | None = None, repeat: int = 1):
    """repeat>1 is a timing mode: the loop runs nsteps*repeat steps; stores
    for t >= nsteps fall out of range and are skipped via bounds_check."""
    out_steps = out_steps or nsteps
    nc = bacc.Bacc()
    cb16 = nc.dram_tensor("cb16", [128, CB_COLS], BF16, kind="ExternalInput")
    cf32 = nc.dram_tensor("cf32", [128, CF_COLS], F32, kind="ExternalInput")
    out = nc.dram_tensor("out", [out_steps * BS, I], F32, kind="ExternalOutput")

    with tile.TileContext(nc) as tc:
        with (
            tc.tile_pool(name="consts", bufs=1) as consts,
            tc.tile_pool(name="state", bufs=1) as state,
            tc.tile_pool(name="gact", bufs=3) as gact,
            tc.tile_pool(name="outp", bufs=3) as outp,
            tc.tile_pool(name="psst", bufs=1, space="PSUM") as psst,
            tc.tile_pool(name="pf", bufs=2, space="PSUM") as pfp,
            tc.tile_pool(name="po", bufs=2, space="PSUM") as pop,
        ):
            CB = consts.tile([128, CB_COLS], BF16)
            nc.sync.dma_start(out=CB, in_=cb16[:])
            CF = consts.tile([128, CF_COLS], F32)
            nc.sync.dma_start(out=CF, in_=cf32[:])
            ones = consts.tile([1, HC * BS], BF16)
            nc.vector.memset(ones, 1.0)

            def wg_chunk(m, k):
                o = OFF_WG + (m * NK + k) * 128
                return CB[:, o:o + 128]

            def wfc_chunk(k):
                o = OFF_WFC + k * 128
                return CB[:, o:o + 128]

            fb_r = CB[0:1, OFF_FCBR:OFF_FCBR + 128]
            BB = CF[:, OFF_BB:OFF_BB + 4 * HC * BS].rearrange(
                "p (j cb) -> p j cb", j=4)
            fb_h = CF[:, OFF_FCBH:OFF_FCBH + 1]

            hA = state.tile([128, HC, BS], BF16)
            nc.vector.tensor_copy(
                hA, CB[:, OFF_H0:OFF_H0 + HC * BS].rearrange(
                    "p (c b) -> p c b", c=HC))
            hB = state.tile([128, HC, BS], BF16)
            cT = state.tile([128, HC, BS], F32)
            nc.vector.tensor_copy(
                cT, CF[:, OFF_C0:OFF_C0 + HC * BS].rearrange(
                    "p (c b) -> p c b", c=HC))
            inT = state.tile([128, BS], BF16)
            nc.vector.tensor_copy(inT, CB[:, OFF_XT:OFF_XT + BS])
            # prologue tanh so the ACT table set is loaded on every path into
            # the loop -- otherwise the table-load lands INSIDE the body
            warm = state.tile([128, 1], F32)
            nc.scalar.activation(warm, CF[:, OFF_FCBH:OFF_FCBH + 1], AF.Tanh)

            # persistent per-gate PSUM accumulators [p, h-chunk, b]; prologue
            # dummy matmuls set every element's has_written bit so the
            # steady-state flow (DVE writes bias, matmuls accumulate with
            # start=False on top) works from the first step
            psg = [psst.tile([128, HC, BS], F32, name=f"psg{j}")
                   for j in range(4)]
            for j in range(4):
                nc.tensor.matmul(psg[j].rearrange("p c b -> p (c b)"),
                                 lhsT=ones[:, 0:128], rhs=ones,
                                 start=True, stop=True, skip_group_check=True)

            cTf = cT.rearrange("p c b -> p (c b)")
            psgf = [p.rearrange("p c b -> p (c b)") for p in psg]

            def step(t, h_in, h_out):
                # Per-gate PSUM: psg[j] holds gate j for all 4 H-chunks.
                # DVE pre-writes the bias into the bank; matmuls accumulate
                # on top (start=False, has_written set in prologue).
                # Gate order i, g, f, o so the c/h chain starts early.
                # sigmoid-free: sg(z)=(tanh(z/2)+1)/2, state C=2c, H=2h
                # (W_hh, fc_W host-halved; g-gate weights/bias host-doubled
                # so every gate uses tanh(0.5*psum)).
                th = {}
                for j in (0, 2, 1, 3):
                    nc.vector.tensor_copy(psgf[j], BB[:, j, :])
                    for c in range(HC):
                        m = j * 4 + c
                        for k in (1, 2, 3, 4, 0):
                            mv = inT if k == 0 else h_in[:, k - 1, :]
                            nc.tensor.matmul(
                                psg[j][:, c, :], lhsT=wg_chunk(m, k), rhs=mv,
                                start=False, stop=(k == 0),
                                skip_group_check=True)
                    th_j = gact.tile([128, HC * BS], F32, tag=f"th{j}")
                    nc.scalar.activation(th_j, psgf[j], AF.Tanh, scale=0.5)
                    th[j] = th_j
                # A=(th_f+1)*C=4fc, B=(th_i+1)*g=2ig, C_new=A/2+B=2c_new
                v_s = gact.tile([128, HC * BS], F32, tag="v_s")
                u_s = gact.tile([128, HC * BS], F32, tag="u_s")
                nc.vector.scalar_tensor_tensor(
                    v_s, th[0], 1.0, th[2],
                    op0=mybir.AluOpType.add, op1=mybir.AluOpType.mult)
                nc.vector.scalar_tensor_tensor(
                    u_s, th[1], 1.0, cTf,
                    op0=mybir.AluOpType.add, op1=mybir.AluOpType.mult)
                nc.vector.scalar_tensor_tensor(
                    cTf, u_s, 0.5, v_s,
                    op0=mybir.AluOpType.mult, op1=mybir.AluOpType.add)
                tc_s = gact.tile([128, HC * BS], F32, tag="tc_s")
                nc.scalar.activation(tc_s, cTf, AF.Tanh, scale=0.5)
                # H = (th_o+1)*tanh(c) = 2h
                nc.vector.scalar_tensor_tensor(
                    h_out.rearrange("p c b -> p (c b)"), th[3], 1.0, tc_s,
                    op0=mybir.AluOpType.add, op1=mybir.AluOpType.mult)

                # feedback fc: inT = tanh(0.5*fc(h) + fc_b/2)  [128 i, BS b]
                pf = pfp.tile([128, BS], F32, tag="pf")
                for k in range(HC):
                    nc.tensor.matmul(pf, lhsT=wfc_chunk(k), rhs=h_out[:, k, :],
                                     start=(k == 0), stop=(k == HC - 1))
                nc.scalar.activation(inT, pf, AF.Tanh, bias=fb_h, scale=0.5)

                # output fc in [b, i] layout for clean DMA; bias via K=1 matmul
                po = pop.tile([BS, 128], F32, tag="po")
                for k in range(HC):
                    nc.tensor.matmul(po, lhsT=h_out[:, k, :], rhs=wfc_chunk(k),
                                     start=(k == 0), stop=False)
                nc.tensor.matmul(po, lhsT=ones[:, 0:BS], rhs=fb_r,
                                 start=False, stop=True)
                ob = outp.tile([BS, 128], F32, tag="ob")
                nc.scalar.activation(ob, po, AF.Tanh, scale=0.5)
                # repeat>1 (timing mode): extra steps clamp to row 0 (junk)
                row = (nsteps - 1 - t) * BS
                if repeat > 1:
                    row = smax(0, row)
                nc.sync.dma_start(out=out[ds(row, BS), :], in_=ob)

            with tc.For_i(0, nsteps * repeat, 2, staggered_reset=True) as t:
                step(t, hA, hB)
                step(t + 1, hB, hA)

    nc.finalize()
    return nc


_cache = {}


def _get_nc(nsteps, out_steps=None, repeat=1):
    key = (nsteps, out_steps, repeat)
    if key not in _cache:
        _cache[key] = build(nsteps, out_steps, repeat)
    return _cache[key]


def _prep_inputs(x, h0, c0, W_ih, W_hh, b_ih, b_hh, fc_W, fc_b, nsteps):
    f32 = np.float32
    bf16 = ml_dtypes.bfloat16
    x = np.asarray(x, f32)
    h0 = np.asarray(h0, f32)
    c0 = np.asarray(c0, f32)
    # state is H=2h, C=2c with W_hh/fc_W halved to compensate; g-gate rows
    # doubled so all gates share tanh(0.5*(psum)) with psum pre-biased
    W_cat = np.concatenate(
        [np.asarray(W_ih, f32), 0.5 * np.asarray(W_hh, f32)], axis=1)
    W_cat[1024:1536, :] *= 2.0
    wg_np = W_cat.reshape(NM, 128, NK, 128).transpose(3, 0, 2, 1).reshape(
        128, NM * NK * 128)
    fc_W = np.asarray(fc_W, f32)
    wfc_np = (0.5 * fc_W).reshape(I, HC, 128).transpose(2, 1, 0).reshape(
        128, HC * 128)
    b = np.asarray(b_ih, f32) + np.asarray(b_hh, f32)
    badj = b.copy()
    badj[1024:1536] *= 2.0
    # bias broadcast [p, gate j, h-chunk c, b] -> [128, 4*HC*BS]
    bb_np = np.broadcast_to(
        badj.reshape(4, HC, 128).transpose(2, 0, 1)[:, :, :, None],
        (128, 4, HC, BS)).reshape(128, 4 * HC * BS)
    fc_b = np.asarray(fc_b, f32)

    cf = np.zeros((128, CF_COLS), f32)
    cf[:, OFF_BB:OFF_BB + 4 * HC * BS] = bb_np
    cf[:, OFF_FCBH] = 0.5 * fc_b

    cb_common = np.zeros((128, CB_COLS), f32)
    cb_common[:, OFF_WG:OFF_WG + NM * NK * 128] = wg_np
    cb_common[:, OFF_WFC:OFF_WFC + HC * 128] = wfc_np
    cb_common[0, OFF_FCBR:OFF_FCBR + 128] = fc_b

    in_maps = []
    for core in range(NCORES):
        sl = slice(core * BS, (core + 1) * BS)
        cb = cb_common.copy()
        cb[:, OFF_XT:OFF_XT + BS] = x[nsteps - 1, sl, :].T
        cb[:, OFF_H0:OFF_H0 + HC * BS] = 2.0 * \
            h0[0, sl, :].reshape(BS, HC, 128).transpose(2, 1, 0).reshape(128, -1)
        cfc = cf.copy()
        cfc[:, OFF_C0:OFF_C0 + HC * BS] = 2.0 * \
            c0[0, sl, :].reshape(BS, HC, 128).transpose(2, 1, 0).reshape(128, -1)
        in_maps.append({
            "cb16": np.ascontiguousarray(cb).astype(bf16),
            "cf32": np.ascontiguousarray(cfc),
        })
    return in_maps


def run(x, h0, c0, W_ih, W_hh, b_ih, b_hh, fc_W, fc_b, nsteps=T,
        out_steps=None, repeat=1, **spmd_kwargs):
    nc = _get_nc(nsteps, out_steps, repeat)
    in_maps = _prep_inputs(x, h0, c0, W_ih, W_hh, b_ih, b_hh, fc_W, fc_b, nsteps)
    res = run_bass_kernel_spmd(nc, in_maps, core_ids=list(range(NCORES)),
                               **spmd_kwargs)
    outs = np.concatenate(
        [r["out"].reshape(out_steps or nsteps, BS, I) for r in res.results],
        axis=1,
    )
    return outs, res


def kernel(x, enc_hiddens, h0, c0, W_ih, W_hh, b_ih, b_hh, fc_W, fc_b):
    outs, _ = run(x, h0, c0, W_ih, W_hh, b_ih, b_hh, fc_W, fc_b, nsteps=T)
    return outs.astype(np.float32)

